# revision 1
# baseline (speedup 1.0000x reference)
"""CoAttention layer kernel for 8 Trainium2 NeuronCores.

Sharding: data-parallel over batch B=16 -> 2 batches per core; all weights
replicated. Per core:
  stage A: load + PE-transpose weights/x/s into [contraction-on-partition] layouts
  stage B: Wx^T = W @ x^T, Us^T = U @ s^T (f32r matmuls), cast fp16
  stage C: e = tanh(Wx + Us) on [e-chunk, (n,s,l)] fp16 tiles (ACT-bound),
           scores = v . e via PE with v embedded at column 32*r of a zero lhsT
           so 4 (b,n) rows land on psum partitions {0,32,64,96}
  stage D: softmax over l + mean over s -> abar
  stage E: u^T = x^T @ abar per (b,n)
  stage F: BiLSTM over n (batched over both dirs + both batches per step)
  stage G: self-attention pooling -> g [2, 256]
"""

import contextlib

import numpy as np

import concourse.bass as bass
import concourse.tile as tile
from concourse import mybir
from concourse.bass_utils import run_bass_kernel_spmd
from concourse.masks import make_identity

F16 = mybir.dt.float16
F32 = mybir.dt.float32
F32R = mybir.dt.float32r
AF = mybir.ActivationFunctionType
AX = mybir.AxisListType

NCORES = 8
B, N, L, S, D, H = 16, 8, 64, 32, 256, 128
BL = B // NCORES  # 2 batches per core
DC = D // 128  # 2 contraction chunks
import os
NPT = int(os.environ.get("K_NPT", "2"))  # n's per e-tile
EFREE = NPT * S * L
NGP = int(os.environ.get("K_NGP", "0"))  # how many e-adds go to gpsimd
PSM_BUFS = int(os.environ.get("K_PSM", "2"))
PT_BUFS = int(os.environ.get("K_PT", "5"))
PE_BUFS = int(os.environ.get("K_PE", "3"))
STOP = os.environ.get("K_STOP", "")


def split_sync_waits(nc, max_waits=1):
    """Walrus codegen caps sync-wait commands per instruction; hoist excess
    waits onto wait-only EventSemaphore carriers inserted just before."""
    for fn in nc.m.functions:
        for blk in fn.blocks:
            out, changed = [], False
            for inst in list(blk.instructions):
                si = inst.sync_info
                if si is not None and len(si.on_wait) > max_waits:
                    waits = list(si.on_wait)
                    extra, keep = waits[:-max_waits], waits[-max_waits:]
                    for i in range(0, len(extra), max_waits):
                        car = mybir.InstEventSemaphore(
                            name=nc.get_next_instruction_name(), ins=[], outs=[]
                        )
                        car.engine = inst.engine
                        car.sync_info = mybir.SyncInfo(
                            on_wait=extra[i : i + max_waits], on_update=[]
                        )
                        out.append(car)
                    inst.sync_info = mybir.SyncInfo(
                        on_wait=keep, on_update=list(si.on_update)
                    )
                    changed = True
                out.append(inst)
            if changed:
                blk.instructions = out


def vw(src, dims):
    """View of sliced AP `src` with free dims replaced by [step, count] list."""
    return bass.AP(tensor=src.tensor, offset=src.offset, ap=[src.ap[0]] + dims)


def build():
    nc = bass.Bass("TRN2", target_bir_lowering=False, debug=False, num_devices=NCORES)

    x_d = nc.dram_tensor("x", [BL, N, L, D], F32, kind="ExternalInput")
    s_d = nc.dram_tensor("s", [BL, S, D], F32, kind="ExternalInput")
    W_d = nc.dram_tensor("W", [D, D], F32, kind="ExternalInput")
    U_d = nc.dram_tensor("U", [D, D], F32, kind="ExternalInput")
    v_d = nc.dram_tensor("v", [D], F32, kind="ExternalInput")
    wih = [
        nc.dram_tensor("Wih_f", [4 * H, D], F32, kind="ExternalInput"),
        nc.dram_tensor("Wih_b", [4 * H, D], F32, kind="ExternalInput"),
    ]
    whh = [
        nc.dram_tensor("Whh_f", [4 * H, H], F32, kind="ExternalInput"),
        nc.dram_tensor("Whh_b", [4 * H, H], F32, kind="ExternalInput"),
    ]
    bih = [
        nc.dram_tensor("bih_f", [4 * H], F32, kind="ExternalInput"),
        nc.dram_tensor("bih_b", [4 * H], F32, kind="ExternalInput"),
    ]
    bhh = [
        nc.dram_tensor("bhh_f", [4 * H], F32, kind="ExternalInput"),
        nc.dram_tensor("bhh_b", [4 * H], F32, kind="ExternalInput"),
    ]
    saW_d = nc.dram_tensor("saW", [D, D], F32, kind="ExternalInput")
    sab_d = nc.dram_tensor("sab", [D], F32, kind="ExternalInput")
    sav_d = nc.dram_tensor("sav", [D], F32, kind="ExternalInput")
    g_d = nc.dram_tensor("g", [BL, D], F32, kind="ExternalOutput")

    with tile.TileContext(nc) as tc, contextlib.ExitStack() as ctx:
        sing = ctx.enter_context(tc.tile_pool(name="sing", bufs=1))
        pe_pool = ctx.enter_context(tc.tile_pool(name="epool", bufs=PE_BUFS))
        pt_pool = ctx.enter_context(tc.tile_pool(name="tpool", bufs=PT_BUFS))
        scq_pool = ctx.enter_context(tc.tile_pool(name="scq", bufs=int(os.environ.get("K_SCQ", "3"))))
        lsm = ctx.enter_context(tc.tile_pool(name="lsm", bufs=int(os.environ.get("K_LSM", "3"))))
        ps_misc = ctx.enter_context(tc.tile_pool(name="psm", bufs=PSM_BUFS, space="PSUM"))
        ps_sc = ctx.enter_context(tc.tile_pool(name="pssc", bufs=int(os.environ.get("K_PSSC", "2")), space="PSUM"))
        ps_u = ctx.enter_context(tc.tile_pool(name="psu", bufs=1, space="PSUM"))
        ps_gh = ctx.enter_context(tc.tile_pool(name="psgh", bufs=int(os.environ.get("K_PSGH", "2")), space="PSUM"))

        ident = sing.tile([128, 128], F32, tag="ident")
        make_identity(nc, ident[:])

        # ---------------- stage A: loads ----------------
        x_nat = sing.tile([64, BL * N * D], F32, tag="x_nat")
        XG = int(os.environ.get("K_XG", "8"))  # n's per x DMA
        for b in range(BL):
            for g in range(N // XG):
                nc.sync.dma_start(
                    x_nat[
                        :, (b * N + g * XG) * D : (b * N + (g + 1) * XG) * D
                    ].rearrange("p (n d) -> p n d", n=XG),
                    x_d.ap()[b, g * XG : (g + 1) * XG].rearrange("n l d -> l n d"),
                )
        s_nat = sing.tile([64, D], F32, tag="s_nat")
        for b in range(BL):
            nc.sync.dma_start(s_nat[b * S : (b + 1) * S, :], s_d.ap()[b])

        W_sb = sing.tile([128, 2 * D], F32, tag="W_sb")  # chunk ec at col ec*D
        U_sb = sing.tile([128, 2 * D], F32, tag="U_sb")
        saW_sb = sing.tile([128, 2 * D], F32, tag="saW_sb")
        for ec in range(2):
            nc.sync.dma_start(W_sb[:, ec * D : (ec + 1) * D], W_d.ap()[bass.ts(ec, 128), :])
            nc.sync.dma_start(U_sb[:, ec * D : (ec + 1) * D], U_d.ap()[bass.ts(ec, 128), :])
            nc.sync.dma_start(
                saW_sb[:, ec * D : (ec + 1) * D], saW_d.ap()[bass.ts(ec, 128), :]
            )
        # small vectors -> [128, k] with strided DMA (col g = elems g*128..)
        def load_cols(dram_ap, ncols, tag):
            t = sing.tile([128, ncols], F32, tag=tag)
            nc.sync.dma_start(t[:], dram_ap.rearrange("(g p) -> p g", p=128))
            return t

        v_sb = load_cols(v_d.ap(), 2, "v_sb")
        sav_sb = load_cols(sav_d.ap(), 2, "sav_sb")
        sab_sb = load_cols(sab_d.ap(), 2, "sab_sb")
        v_h = sing.tile([128, 2], F16, tag="v_h")
        nc.vector.tensor_copy(v_h[:], v_sb[:])
        # vblk[ec]: [128, 32] zeros with v_h[:, ec] in col 0
        vblk = []
        for ec in range(2):
            vb = sing.tile([128, 32], F16, tag=f"vblk{ec}")
            nc.gpsimd.memset(vb[:], 0.0)
            nc.vector.tensor_copy(vb[:, 0:1], v_h[:, ec : ec + 1])
            vblk.append(vb)

        # ---------------- stage A: transposes ----------------
        def transpose_blocks(dst, srcs, copy_engine=None):
            """srcs: list of (in_ap [p<=128, 128], out_col). dst: sbuf tile."""
            ncols = sum(s[0].shape[0] for s in srcs)
            pst = ps_misc.tile([128, 512], F32, tag="tr")
            col = 0
            for in_ap, _ in srcs:
                k = in_ap.shape[0]
                nc.tensor.transpose(pst[:, col : col + k], in_ap, ident[0:k, 0:k])
                col += k
            eng = copy_engine or nc.vector
            eng.tensor_copy(dst, pst[:, 0:ncols])

        WT = [sing.tile([128, D], F32, tag=f"WT{dc}", name=f"WT{dc}") for dc in range(DC)]
        UT = [sing.tile([128, D], F32, tag=f"UT{dc}", name=f"UT{dc}") for dc in range(DC)]
        saWT = [sing.tile([128, D], F16, tag=f"saWT{dc}", name=f"saWT{dc}") for dc in range(DC)]
        for dc in range(DC):
            for src, dstl in ((W_sb, WT), (U_sb, UT)):
                transpose_blocks(
                    dstl[dc][:],
                    [
                        (src[:, ec * D + dc * 128 : ec * D + (dc + 1) * 128], ec * 128)
                        for ec in range(2)
                    ],
                )
        xT = [sing.tile([128, BL * N * L], F32, tag=f"xT{dc}", name=f"xT{dc}") for dc in range(DC)]
        for b in range(BL):
            for dc in range(DC):
                transpose_blocks(
                    xT[dc][:, b * N * L : (b + 1) * N * L],
                    [
                        (
                            x_nat[
                                0:64,
                                (b * N + n) * D + dc * 128 : (b * N + n) * D
                                + (dc + 1) * 128,
                            ],
                            n * 64,
                        )
                        for n in range(N)
                    ],
                )
        sT = [sing.tile([128, BL * S], F32, tag=f"sT{dc}", name=f"sT{dc}") for dc in range(DC)]
        for dc in range(DC):
            transpose_blocks(sT[dc][:], [(s_nat[0:64, dc * 128 : (dc + 1) * 128], 0)])

        # ---------------- stage B: Wx^T, Us^T (fp16) ----------------
        WxT_h = [sing.tile([128, BL * N * L], F16, tag=f"WxT{ec}", name=f"WxT{ec}") for ec in range(2)]
        for b in range(BL):
            for ec in range(2):
                pst = ps_misc.tile([128, 512], F32, tag="tr")
                for dc in range(DC):
                    nc.tensor.matmul(
                        pst[:],
                        WT[dc][:, ec * 128 : (ec + 1) * 128],
                        xT[dc][:, b * 512 : (b + 1) * 512],
                        start=(dc == 0),
                        stop=(dc == DC - 1),
                    )
                nc.vector.tensor_copy(WxT_h[ec][:, b * 512 : (b + 1) * 512], pst[:])
        UsT_h = [sing.tile([128, BL * S], F16, tag=f"UsT{ec}", name=f"UsT{ec}") for ec in range(2)]
        for ec in range(2):
            pst = ps_misc.tile([128, 512], F32, tag="tr")
            for dc in range(DC):
                nc.tensor.matmul(
                    pst[:, 0 : BL * S],
                    UT[dc][:, ec * 128 : (ec + 1) * 128],
                    sT[dc][:],
                    start=(dc == 0),
                    stop=(dc == DC - 1),
                )
            nc.vector.tensor_copy(UsT_h[ec][:], pst[:, 0 : BL * S])
        # expand Us along l once per (b, ec)
        us_x = {}
        for b in range(BL):
            for ec in range(2):
                t = sing.tile([128, S * L], F16, tag=f"usx{b}{ec}")
                nc.vector.tensor_copy(
                    t[:].rearrange("p (s l) -> p s l", s=S),
                    vw(UsT_h[ec][:, b * S : (b + 1) * S], [[1, S], [0, L]]),
                )
                us_x[(b, ec)] = t

        if STOP == "B":
            g_sb = sing.tile([BL, D], F32, tag="g_sb")
            nc.vector.tensor_copy(g_sb[:], WxT_h[0][0:BL, 0:D])
            nc.sync.dma_start(g_d.ap()[:], g_sb[:])
            split_sync_waits(nc)
            return nc

        # ---------------- stages C+D+E ----------------
        scores_sb = sing.tile([64, S * L], F32, tag="scores")
        for b in range(BL):
            for nq in range(N // NPT):
                tt = {}
                for ec in range(2):
                    e_t = pe_pool.tile([128, EFREE], F16, tag="e")
                    in0 = vw(
                        WxT_h[ec][:, b * N * L + nq * NPT * L :],
                        [[L, NPT], [0, S], [1, L]],
                    )
                    in1 = vw(us_x[(b, ec)][:], [[0, NPT], [L, S], [1, L]])
                    eng = nc.gpsimd if (b * (N // NPT) + nq) * 2 + ec < NGP else nc.vector
                    eng.tensor_add(
                        e_t[:].rearrange("p (a s l) -> p a s l", a=NPT, s=S), in0, in1
                    )
                    t_t = pt_pool.tile([128, EFREE], F16, tag="t")
                    nc.scalar.activation(t_t[:], e_t[:], AF.Tanh)
                    tt[ec] = t_t
                # scores: quad r = n offset nq*NPT + r
                for j in range(S * L // 512):
                    scp = ps_sc.tile([128, 512], F32, tag="sc")
                    for r in range(NPT):
                        for ec in range(2):
                            nc.tensor.matmul(
                                scp[32 * r : 32 * r + 32, :],
                                vblk[ec][:],
                                tt[ec][:, r * S * L + j * 512 : r * S * L + (j + 1) * 512],
                                start=(ec == 0),
                                stop=(ec == 1),
                                tile_position=(0, 32 * r),
                            )
                    scq = scq_pool.tile([128, 512], F32, tag="scq")
                    nc.vector.tensor_copy(scq[:], scp[:])
                    for r in range(NPT):
                        row = b * 32 + nq * NPT + r
                        nc.sync.dma_start(
                            scores_sb[row : row + 1, j * 512 : (j + 1) * 512],
                            scq[32 * r : 32 * r + 1, :],
                        )

        if STOP == "C":
            g_sb = sing.tile([BL, D], F32, tag="g_sb")
            nc.vector.tensor_copy(g_sb[:], scores_sb[0:BL, 0:D])
            nc.sync.dma_start(g_d.ap()[:], g_sb[:])
            split_sync_waits(nc)
            return nc

        # stage D: softmax over l, mean over s -> abar [16, L] (per-b to overlap C)
        SM = nc.gpsimd if os.environ.get("K_SMG", "0") == "1" else nc.vector
        abar = sing.tile([64, L], F32, tag="abar")
        mx = sing.tile([64, S], F32, tag="mx")
        es = sing.tile([64, S * L], F32, tag="es")
        den = sing.tile([64, S], F32, tag="den")
        rden = sing.tile([64, S], F32, tag="rden")
        for b in range(BL):
            r0, r1 = b * 32, b * 32 + N
            sco = scores_sb[r0:r1, :].rearrange("p (s l) -> p s l", s=S)
            if os.environ.get("K_MAXSUB", "0") == "1":
                nc.vector.reduce_max(mx[r0:r1, :], sco, axis=AX.X)
                SM.tensor_sub(
                    es[r0:r1, :].rearrange("p (s l) -> p s l", s=S),
                    sco,
                    vw(mx[r0:r1, :][:, 0:1], [[1, S], [0, L]]),
                )
                nc.scalar.activation(es[r0:r1, :], es[r0:r1, :], AF.Exp)
            else:
                # scores bounded (|v . tanh| small): exp directly, softmax is
                # shift-invariant so skipping the max changes nothing in f32
                nc.scalar.activation(es[r0:r1, :], scores_sb[r0:r1, :], AF.Exp)
            nc.vector.reduce_sum(
                den[r0:r1, :],
                es[r0:r1, :].rearrange("p (s l) -> p s l", s=S),
                axis=AX.X,
            )
            nc.vector.reciprocal(rden[r0:r1, :], den[r0:r1, :])
            nc.vector.tensor_scalar_mul(rden[r0:r1, :], rden[r0:r1, :], 1.0 / S)
            SM.tensor_mul(
                es[r0:r1, :].rearrange("p (s l) -> p s l", s=S),
                es[r0:r1, :].rearrange("p (s l) -> p s l", s=S),
                vw(rden[r0:r1, :][:, 0:1], [[1, S], [0, L]]),
            )
            nc.vector.reduce_sum(
                abar[r0:r1, :], vw(es[r0:r1, :][:, 0:1], [[1, L], [L, S]]), axis=AX.X
            )

        # stage E: u^T[d, bn] = x^T @ abar
        abarT = sing.tile([64, BL * N], F32, tag="abarT")
        pst = ps_misc.tile([128, 512], F32, tag="tr")
        for b in range(BL):
            nc.tensor.transpose(
                pst[0:L, b * N : (b + 1) * N], abar[b * 32 : b * 32 + N, :],
                ident[b * 32 : b * 32 + N, b * 32 : b * 32 + N],
            )
        nc.vector.tensor_copy(abarT[:], pst[0:L, 0 : BL * N])
        uT = [sing.tile([128, BL * N], F16, tag=f"uT{dc}", name=f"uT{dc}") for dc in range(DC)]
        for dc in range(DC):
            up = ps_u.tile([128, BL * N], F32, tag="up")
            for bn in range(BL * N):
                nc.tensor.matmul(
                    up[:, bn : bn + 1],
                    x_nat[0:64, bn * D + dc * 128 : bn * D + (dc + 1) * 128],
                    abarT[0:L, bn : bn + 1],
                    start=True,
                    stop=True,
                )
            nc.vector.tensor_copy(uT[dc][:], up[:])

        wih_sb = []  # [dir] tile [128, 4*D], gc chunk at col gc*D
        whh_sb = []  # [dir] tile [128, 4*H]
        for dr in range(2):
            wt = sing.tile([128, 4 * D], F32, tag=f"wih_sb{dr}")
            for gc in range(4):
                nc.sync.dma_start(
                    wt[:, gc * D : (gc + 1) * D], wih[dr].ap()[bass.ts(gc, 128), :]
                )
            wih_sb.append(wt)
            ht = sing.tile([128, 4 * H], F32, tag=f"whh_sb{dr}")
            for gc in range(4):
                nc.sync.dma_start(
                    ht[:, gc * H : (gc + 1) * H], whh[dr].ap()[bass.ts(gc, 128), :]
                )
            whh_sb.append(ht)

        bias_sb = []
        for dr in range(2):
            b1 = load_cols(bih[dr].ap(), 4, f"bih{dr}")
            b2 = load_cols(bhh[dr].ap(), 4, f"bhh{dr}")
            bt = sing.tile([128, 4], F32, tag=f"bias{dr}")
            nc.vector.tensor_add(bt[:], b1[:], b2[:])
            bias_sb.append(bt)

        wihT = []  # [dir][dc] [128, 512]
        for dr in range(2):
            row = []
            for dc in range(DC):
                dst = sing.tile([128, 4 * H], F16, tag=f"wihT{dr}{dc}")
                transpose_blocks(
                    dst[:],
                    [
                        (
                            wih_sb[dr][:, gc * D + dc * 128 : gc * D + (dc + 1) * 128],
                            gc * 128,
                        )
                        for gc in range(4)
                    ],
                )
                row.append(dst)
            wihT.append(row)
        whhT = []
        for dr in range(2):
            dst = sing.tile([128, 4 * H], F16, tag=f"whhT{dr}")
            transpose_blocks(
                dst[:],
                [(whh_sb[dr][:, gc * H : (gc + 1) * H], gc * 128) for gc in range(4)],
            )
            whhT.append(dst)


        if STOP == "E":
            g_sb = sing.tile([BL, D], F32, tag="g_sb")
            nc.vector.tensor_copy(g_sb[:], uT[0][0:BL, 0:16])
            nc.sync.dma_start(g_d.ap()[:, 0:16], g_sb[:, 0:16])
            split_sync_waits(nc)
            return nc

        # ---------------- stage F: BiLSTM ----------------
        # gate slots within gxb cols: i->0, f->1, o->2, g->3 (pytorch gc order i,f,g,o)
        SLOT = {0: 0, 1: 1, 2: 3, 3: 2}
        gxb = sing.tile([128, 128], F32, tag="gxb")  # dir*64 + slot*16 + b*8 + n
        for dr in range(2):
            gp = ps_misc.tile([128, 512], F32, tag="tr")
            for gc in range(4):
                sl = SLOT[gc]
                for dc in range(DC):
                    nc.tensor.matmul(
                        gp[:, sl * 16 : (sl + 1) * 16],
                        wihT[dr][dc][:, gc * 128 : (gc + 1) * 128],
                        uT[dc][:],
                        start=(dc == 0),
                        stop=(dc == DC - 1),
                    )
            for gc in range(4):
                sl = SLOT[gc]
                nc.vector.tensor_scalar_add(
                    gxb[:, dr * 64 + sl * 16 : dr * 64 + (sl + 1) * 16],
                    gp[:, sl * 16 : (sl + 1) * 16],
                    bias_sb[dr][:, gc : gc + 1],
                )

        zeros4 = sing.tile([128, 4], F32, tag="zeros4")
        nc.gpsimd.memset(zeros4[:], 0.0)
        hseqT = [sing.tile([128, BL * N], F16, tag=f"hseqT{dr}", name=f"hseqT{dr}") for dr in range(2)]
        zeros_h = sing.tile([128, 2], F16, tag="zeros_h")
        nc.gpsimd.memset(zeros_h[:], 0.0)
        h_prev = [zeros_h[:], zeros_h[:]]
        c_prev = zeros4[:]
        # gsum cols: dirF sig i0:2 f2:4 o4:6 | dirB sig 6:12 | gF 12:14 gB 14:16
        for k in range(N):
            ghp = ps_gh.tile([128, 16], F32, tag="gh")
            for dr in range(2):
                base = dr * 6
                for gc, col in ((0, base), (1, base + 2), (3, base + 4), (2, 12 + dr * 2)):
                    nc.tensor.matmul(
                        ghp[:, col : col + 2],
                        whhT[dr][:, gc * 128 : (gc + 1) * 128],
                        h_prev[dr],
                        start=True,
                        stop=True,
                    )
            gsum = lsm.tile([128, 16], F32, tag="gsum")
            pf, pb = k, N - 1 - k
            if pf == pb:
                gx_sig = vw(gxb[:, pf : pf + 1], [[64, 2], [16, 3], [8, 2]])
                gx_tan = vw(gxb[:, 48 + pf : 48 + pf + 1], [[64, 2], [8, 2]])
            else:
                gx_sig = vw(
                    gxb[:, pf : pf + 1], [[64 + pb - pf, 2], [16, 3], [8, 2]]
                )
                gx_tan = vw(
                    gxb[:, 48 + pf : 48 + pf + 1], [[64 + pb - pf, 2], [8, 2]]
                )
            nc.vector.tensor_add(
                gsum[:, 0:12].rearrange("p (d g b) -> p d g b", d=2, g=3),
                ghp[:, 0:12].rearrange("p (d g b) -> p d g b", d=2, g=3),
                gx_sig,
            )
            nc.vector.tensor_add(
                gsum[:, 12:16].rearrange("p (d b) -> p d b", d=2),
                ghp[:, 12:16].rearrange("p (d b) -> p d b", d=2),
                gx_tan,
            )
            sg = lsm.tile([128, 12], F32, tag="sg")
            nc.scalar.activation(sg[:], gsum[:, 0:12], AF.Sigmoid)
            tg = lsm.tile([128, 4], F32, tag="tg")
            nc.scalar.activation(tg[:], gsum[:, 12:16], AF.Tanh)
            t1 = lsm.tile([128, 4], F32, tag="t1")
            nc.vector.tensor_mul(t1[:], vw(sg[:, 2:3], [[6, 2], [1, 2]]), c_prev)
            t2 = lsm.tile([128, 4], F32, tag="t2")
            nc.gpsimd.tensor_mul(t2[:], vw(sg[:, 0:1], [[6, 2], [1, 2]]), tg[:])
            c_new = lsm.tile([128, 4], F32, tag="c")
            nc.vector.tensor_add(c_new[:], t1[:], t2[:])
            tc_ = lsm.tile([128, 4], F32, tag="tc")
            nc.scalar.activation(tc_[:], c_new[:], AF.Tanh)
            for dr in range(2):
                pos = k if dr == 0 else N - 1 - k
                hv = vw(hseqT[dr][:, pos : pos + 1], [[8, 2]])
                nc.vector.tensor_mul(
                    hv, sg[:, 4 + dr * 6 : 6 + dr * 6], tc_[:, dr * 2 : dr * 2 + 2]
                )
                h_prev[dr] = hv
            c_prev = c_new[:]

        for dc in range(DC):
            transpose_blocks(
                saWT[dc][:],
                [
                    (
                        saW_sb[:, ec * D + dc * 128 : ec * D + (dc + 1) * 128],
                        ec * 128,
                    )
                    for ec in range(2)
                ],
            )

        if STOP == "F":
            g_sb = sing.tile([BL, D], F32, tag="g_sb")
            nc.vector.tensor_copy(g_sb[:, 0:16], hseqT[0][0:BL, :])
            nc.sync.dma_start(g_d.ap()[:, 0:16], g_sb[:, 0:16])
            split_sync_waits(nc)
            return nc

        # ---------------- stage G: attention pooling ----------------
        th = []
        for ec in range(2):
            ap_ps = ps_misc.tile([128, 512], F32, tag="tr")
            for dc in range(DC):
                nc.tensor.matmul(
                    ap_ps[:, 0 : BL * N],
                    saWT[dc][:, ec * 128 : (ec + 1) * 128],
                    hseqT[dc][:],
                    start=(dc == 0),
                    stop=(dc == DC - 1),
                )
            t = sing.tile([128, BL * N], F32, tag=f"th{ec}")
            nc.scalar.activation(
                t[:], ap_ps[:, 0 : BL * N], AF.Tanh, bias=sab_sb[:, ec : ec + 1]
            )
            th.append(t)
        att_ps = ps_gh.tile([1, BL * N], F32, tag="gh")
        for ec in range(2):
            nc.tensor.matmul(
                att_ps[:], sav_sb[:, ec : ec + 1], th[ec][:], start=(ec == 0), stop=(ec == 1)
            )
        att_sb = sing.tile([1, BL * N], F32, tag="att_sb")
        nc.vector.tensor_copy(att_sb[:], att_ps[:])
        attv = att_sb[:].rearrange("p (b n) -> p b n", b=BL)
        mxa = sing.tile([1, BL], F32, tag="mxa")
        nc.vector.reduce_max(mxa[:], attv, axis=AX.X)
        nc.vector.tensor_sub(attv, attv, vw(mxa[:], [[1, BL], [0, N]]))
        nc.scalar.activation(att_sb[:], att_sb[:], AF.Exp)
        dena = sing.tile([1, BL], F32, tag="dena")
        nc.vector.reduce_sum(dena[:], attv, axis=AX.X)
        rdena = sing.tile([1, BL], F32, tag="rdena")
        nc.vector.reciprocal(rdena[:], dena[:])
        nc.vector.tensor_mul(attv, attv, vw(rdena[:], [[1, BL], [0, N]]))
        # transpose attw [1,16] -> [16,1]
        attT = sing.tile([BL * N, 1], F32, tag="attT")
        pst = ps_misc.tile([128, 512], F32, tag="tr")
        nc.tensor.transpose(pst[0 : BL * N, 0:1], att_sb[:], ident[0:1, 0:1])
        nc.vector.tensor_copy(attT[:], pst[0 : BL * N, 0:1])
        # c_nat [16, 256]
        c_nat = sing.tile([BL * N, D], F32, tag="c_nat")
        ident_h = sing.tile([128, 128], F16, tag="ident_h")
        nc.vector.tensor_copy(ident_h[:], ident[:])
        for dc in range(DC):
            pst = ps_misc.tile([128, 512], F16, tag="tr")
            nc.tensor.transpose(pst[0 : BL * N, 0:128], hseqT[dc][:], ident_h[:, :])
            nc.vector.tensor_copy(
                c_nat[:, dc * 128 : (dc + 1) * 128], pst[0 : BL * N, 0:128]
            )
        cw = sing.tile([BL * N, D], F32, tag="cw")
        nc.vector.tensor_scalar_mul(cw[:], c_nat[:], attT[:, 0:1])
        # ones_blk[p, c] = 1 iff p // N == c  (block-diagonal batch mask)
        ones_blk = sing.tile([BL * N, BL], F32, tag="ones_blk")
        nc.gpsimd.memset(ones_blk[:], 1.0)
        nc.gpsimd.affine_select(
            out=ones_blk[:], in_=ones_blk[:], pattern=[[N, BL]],
            channel_multiplier=-1, base=N - 1, compare_op=mybir.AluOpType.is_ge,
            fill=0.0,
        )
        nc.gpsimd.affine_select(
            out=ones_blk[:], in_=ones_blk[:], pattern=[[-N, BL]],
            channel_multiplier=1, base=0, compare_op=mybir.AluOpType.is_ge,
            fill=0.0,
        )
        g_ps = ps_gh.tile([BL, D], F32, tag="gh")
        nc.tensor.matmul(
            g_ps[:], ones_blk[:], cw[:], start=True, stop=True
        )
        g_sb = sing.tile([BL, D], F32, tag="g_sb")
        nc.vector.tensor_copy(g_sb[:], g_ps[:])
        nc.sync.dma_start(g_d.ap()[:], g_sb[:])

    split_sync_waits(nc)
    return nc


_NC = None


def kernel(**inputs) -> np.ndarray:
    global _NC
    if _NC is None:
        _NC = build()
    f32 = lambda a: np.ascontiguousarray(np.asarray(a, dtype=np.float32))
    shared = {
        k: f32(inputs[k])
        for k in (
            "W",
            "U",
            "v",
            "Wih_f",
            "Whh_f",
            "bih_f",
            "bhh_f",
            "Wih_b",
            "Whh_b",
            "bih_b",
            "bhh_b",
            "saW",
            "sab",
            "sav",
        )
    }
    x = f32(inputs["x"])
    s = f32(inputs["s"])
    in_maps = []
    for c in range(NCORES):
        m = dict(shared)
        m["x"] = np.ascontiguousarray(x[c * BL : (c + 1) * BL])
        m["s"] = np.ascontiguousarray(s[c * BL : (c + 1) * BL])
        in_maps.append(m)
    res = run_bass_kernel_spmd(_NC, in_maps, core_ids=list(range(NCORES)))
    return np.concatenate([r["g"] for r in res.results], axis=0)



# revision 6
# speedup vs baseline: 1.2853x; 1.2853x over previous
"""CoAttention layer kernel for 8 Trainium2 NeuronCores.

Sharding: data-parallel over batch B=16 -> 2 batches per core; all weights
replicated. Per core, pipelined over n-pairs p=(p, 7-p):
  prep: single-DMA loads, PE transposes, Wx^T/Us^T (f32r), us_x expansion
  stage C (per pair, per b, per ec): e = Wx + Us (DVE f16), tanh (ACT),
    then 8 score matmuls with shifted-v lhsT slices accumulating rows
    (b,pos,oct) directly into a [16, 512] PSUM tile (no gather DMAs)
  per-pair tail (emitted one pair behind stage C so ACT never stalls):
    exp -> den -> rden -> es*rden -> abar partials (DVE), abar^T + row-sum
    via ones-matmul, u^T per (b,n), gate-x matmuls with bias folded in as a
    rank-1 matmul; LSTM steps run as soon as their n-pair inputs exist
    (pair p unlocks step p; pair 3 unlocks steps 3..7)
  LSTM uses the tanh-half trick: sigmoid(x) = (tanh(x/2)+1)/2 with i,f,o
  weight rows pre-scaled by 0.5, h carried as 2h (whh pre-scaled 0.5x more,
  saW pre-scaled 0.5, att pre-scaled 0.5), c carried doubled into tanh
  with scale=0.5.
"""

import contextlib

import numpy as np

import concourse.bass as bass
import concourse.tile as tile
from concourse import mybir
from concourse.bass_utils import run_bass_kernel_spmd
from concourse.masks import make_identity

F16 = mybir.dt.float16
F32 = mybir.dt.float32
AF = mybir.ActivationFunctionType
AX = mybir.AxisListType
ALU = mybir.AluOpType

NCORES = 8
B, N, L, S, D, H = 16, 8, 64, 32, 256, 128
BL = B // NCORES  # 2 batches per core
DC = D // 128  # 2 contraction chunks
NPAIR = N // 2  # n-pairs (p, 7-p)
SL = S * L  # 2048


def split_sync_waits(nc, max_waits=1):
    """Walrus codegen caps sync-wait commands per instruction; hoist excess
    waits onto wait-only EventSemaphore carriers inserted just before."""
    for fn in nc.m.functions:
        for blk in fn.blocks:
            out, changed = [], False
            for inst in list(blk.instructions):
                si = inst.sync_info
                if si is not None and len(si.on_wait) > max_waits:
                    waits = list(si.on_wait)
                    extra, keep = waits[:-max_waits], waits[-max_waits:]
                    for i in range(0, len(extra), max_waits):
                        car = mybir.InstEventSemaphore(
                            name=nc.get_next_instruction_name(), ins=[], outs=[]
                        )
                        car.engine = inst.engine
                        car.sync_info = mybir.SyncInfo(
                            on_wait=extra[i : i + max_waits], on_update=[]
                        )
                        out.append(car)
                    inst.sync_info = mybir.SyncInfo(
                        on_wait=keep, on_update=list(si.on_update)
                    )
                    changed = True
                out.append(inst)
            if changed:
                blk.instructions = out


def vw(src, dims):
    """View of sliced AP `src` with free dims replaced by [step, count] list."""
    return bass.AP(tensor=src.tensor, offset=src.offset, ap=[src.ap[0]] + dims)


def build():
    nc = bass.Bass("TRN2", target_bir_lowering=False, debug=False, num_devices=NCORES)

    x_d = nc.dram_tensor("x", [BL, N, L, D], F32, kind="ExternalInput")
    s_d = nc.dram_tensor("s", [BL, S, D], F32, kind="ExternalInput")
    W_d = nc.dram_tensor("W", [D, D], F32, kind="ExternalInput")
    U_d = nc.dram_tensor("U", [D, D], F32, kind="ExternalInput")
    v_d = nc.dram_tensor("v", [D], F32, kind="ExternalInput")
    wih = [
        nc.dram_tensor("Wih_f", [4 * H, D], F32, kind="ExternalInput"),
        nc.dram_tensor("Wih_b", [4 * H, D], F32, kind="ExternalInput"),
    ]
    whh = [
        nc.dram_tensor("Whh_f", [4 * H, H], F32, kind="ExternalInput"),
        nc.dram_tensor("Whh_b", [4 * H, H], F32, kind="ExternalInput"),
    ]
    bih = [
        nc.dram_tensor("bih_f", [4 * H], F32, kind="ExternalInput"),
        nc.dram_tensor("bih_b", [4 * H], F32, kind="ExternalInput"),
    ]
    bhh = [
        nc.dram_tensor("bhh_f", [4 * H], F32, kind="ExternalInput"),
        nc.dram_tensor("bhh_b", [4 * H], F32, kind="ExternalInput"),
    ]
    saW_d = nc.dram_tensor("saW", [D, D], F32, kind="ExternalInput")
    sab_d = nc.dram_tensor("sab", [D], F32, kind="ExternalInput")
    sav_d = nc.dram_tensor("sav", [D], F32, kind="ExternalInput")
    g_d = nc.dram_tensor("g", [BL, D], F32, kind="ExternalOutput")

    with tile.TileContext(nc) as tc, contextlib.ExitStack() as ctx:
        sing = ctx.enter_context(tc.tile_pool(name="sing", bufs=1))
        ep = ctx.enter_context(tc.tile_pool(name="ep", bufs=3))
        tp = ctx.enter_context(tc.tile_pool(name="tp", bufs=3))
        esp = ctx.enter_context(tc.tile_pool(name="esp", bufs=2))
        lsm = ctx.enter_context(tc.tile_pool(name="lsm", bufs=3))
        ps_tr = ctx.enter_context(tc.tile_pool(name="pstr", bufs=2, space="PSUM"))
        ps_sc = ctx.enter_context(tc.tile_pool(name="pssc", bufs=2, space="PSUM"))
        ps_sm = ctx.enter_context(tc.tile_pool(name="pssm", bufs=4, space="PSUM"))

        # ---------------- DMAs (one per tensor; order matters) ----------------
        s_nat = sing.tile([BL * S, D], F32, tag="s_nat")  # row = b*S+s
        nc.sync.dma_start(
            s_nat[:].rearrange("p d -> p d"),
            s_d.ap().rearrange("b s d -> (b s) d"),
        )
        W_sb = sing.tile([128, 2 * D], F32, tag="W_sb")  # col = ec*D + d
        U_sb = sing.tile([128, 2 * D], F32, tag="U_sb")
        nc.sync.dma_start(
            W_sb[:].rearrange("p (e d) -> p e d", e=2),
            W_d.ap().rearrange("(e p) d -> p e d", p=128),
        )
        nc.sync.dma_start(
            U_sb[:].rearrange("p (e d) -> p e d", e=2),
            U_d.ap().rearrange("(e p) d -> p e d", p=128),
        )
        x_nat = sing.tile([L, BL * N * D], F32, tag="x_nat")  # col = (b*N+n)*D+d
        for b in range(BL):
            nc.sync.dma_start(
                x_nat[:, b * N * D : (b + 1) * N * D].rearrange(
                    "p (n d) -> p n d", n=N
                ),
                x_d.ap()[b].rearrange("n l d -> l n d"),
            )

        def load_cols(dram_ap, ncols, tag):
            t = sing.tile([128, ncols], F32, tag=tag)
            nc.sync.dma_start(t[:], dram_ap.rearrange("(g p) -> p g", p=128))
            return t

        v_sb = load_cols(v_d.ap(), 2, "v_sb")
        wih_sb = []  # [dr] tile [128, 4*D], gc chunk at col gc*D
        whh_sb = []  # [dr] tile [128, 4*H]
        for dr in range(2):
            wt = sing.tile([128, 4 * D], F32, tag=f"wih_sb{dr}")
            nc.sync.dma_start(
                wt[:].rearrange("p (g d) -> p g d", g=4),
                wih[dr].ap().rearrange("(g p) d -> p g d", p=128),
            )
            wih_sb.append(wt)
            ht = sing.tile([128, 4 * H], F32, tag=f"whh_sb{dr}")
            nc.sync.dma_start(
                ht[:].rearrange("p (g h) -> p g h", g=4),
                whh[dr].ap().rearrange("(g p) h -> p g h", p=128),
            )
            whh_sb.append(ht)
        bias_raw = []
        for dr in range(2):
            b1 = load_cols(bih[dr].ap(), 4, f"bih{dr}")
            b2 = load_cols(bhh[dr].ap(), 4, f"bhh{dr}")
            bias_raw.append((b1, b2))
        saW_sb = sing.tile([128, 2 * D], F32, tag="saW_sb")
        nc.sync.dma_start(
            saW_sb[:].rearrange("p (e d) -> p e d", e=2),
            saW_d.ap().rearrange("(e p) d -> p e d", p=128),
        )
        sab_sb = load_cols(sab_d.ap(), 2, "sab_sb")
        sav_sb = load_cols(sav_d.ap(), 2, "sav_sb")

        ident = sing.tile([128, 128], F32, tag="ident")
        make_identity(nc, ident[:])

        # ---------------- prep: transposes + Wx/Us ----------------
        # sT[dc]: [128(d), BL*S]
        sT = sing.tile([128, 2 * BL * S], F32, tag="sT")  # col = dc*64 + (b,s)
        pst = ps_tr.tile([128, 512], F32, tag="tr")
        for dc in range(DC):
            nc.tensor.transpose(
                pst[:, dc * 64 : dc * 64 + 64],
                s_nat[:, dc * 128 : (dc + 1) * 128],
                ident[0 : BL * S, 0 : BL * S],
            )
        nc.vector.tensor_copy(sT[:], pst[:, 0 : 2 * BL * S])

        # WT[dc]/UT[dc]: [128(d-chunk), 2*128(e)] at col ec*128
        WT = [sing.tile([128, D], F32, tag=f"WT{dc}", name=f"WT{dc}") for dc in range(DC)]
        UT = [sing.tile([128, D], F32, tag=f"UT{dc}", name=f"UT{dc}") for dc in range(DC)]
        for src, dstl in ((W_sb, WT), (U_sb, UT)):
            for dc in range(DC):
                pst = ps_tr.tile([128, 512], F32, tag="tr")
                for ec in range(2):
                    nc.tensor.transpose(
                        pst[:, ec * 128 : (ec + 1) * 128],
                        src[:, ec * D + dc * 128 : ec * D + (dc + 1) * 128],
                        ident[:],
                    )
                nc.vector.tensor_copy(dstl[dc][:], pst[:, 0:D])

        # UsT_h[ec]: [128(e), BL*S] f16
        UsT_h = [sing.tile([128, BL * S], F16, tag=f"UsT{ec}", name=f"UsT{ec}") for ec in range(2)]
        for ec in range(2):
            psu = ps_sm.tile([128, BL * S], F32, tag="sm")
            for dc in range(DC):
                nc.tensor.matmul(
                    psu[:],
                    UT[dc][:, ec * 128 : (ec + 1) * 128],
                    sT[:, dc * 64 : dc * 64 + 2 * BL * S // 2],
                    start=(dc == 0),
                    stop=(dc == DC - 1),
                )
            nc.vector.tensor_copy(UsT_h[ec][:], psu[:])

        # us_x[(b,ec)]: [128, S*L] f16, Us expanded along l (b0 first)
        us_x = {}

        def emit_us_x(b, ec):
            t = sing.tile([128, SL], F16, tag=f"usx{b}{ec}")
            nc.vector.tensor_copy(
                t[:].rearrange("p (s l) -> p s l", s=S),
                vw(UsT_h[ec][:, b * S : (b + 1) * S], [[1, S], [0, L]]),
            )
            us_x[(b, ec)] = t

        emit_us_x(0, 0)

        # xT[b][dc]: [128(d), N*L] f32; WxT_h[ec]: [128(e), BL*N*L] f16
        xT = [[None] * DC for _ in range(BL)]
        WxT_h = [sing.tile([128, BL * N * L], F16, tag=f"WxT{ec}", name=f"WxT{ec}") for ec in range(2)]

        def emit_xT(b):
            for dc in range(DC):
                pst = ps_tr.tile([128, 512], F32, tag="tr")
                for n in range(N):
                    nc.tensor.transpose(
                        pst[:, n * L : (n + 1) * L],
                        x_nat[:, (b * N + n) * D + dc * 128 : (b * N + n) * D + (dc + 1) * 128],
                        ident[0:L, 0:L],
                    )
                t = sing.tile([128, N * L], F32, tag=f"xT{b}{dc}")
                nc.vector.tensor_copy(t[:], pst[:])
                xT[b][dc] = t

        def emit_WxT(b, ec):
            pst = ps_tr.tile([128, 512], F32, tag="tr")
            for dc in range(DC):
                nc.tensor.matmul(
                    pst[:],
                    WT[dc][:, ec * 128 : (ec + 1) * 128],
                    xT[b][dc][:],
                    start=(dc == 0),
                    stop=(dc == DC - 1),
                )
            nc.vector.tensor_copy(WxT_h[ec][:, b * N * L : (b + 1) * N * L], pst[:])

        emit_xT(0)
        emit_WxT(0, 0)
        emit_us_x(0, 1)
        emit_WxT(0, 1)

        # VS[ec]: [128, 31] f16 zeros with v chunk at col 15 (shifted-v lhsT)
        v_h = sing.tile([128, 2], F16, tag="v_h")
        nc.vector.tensor_copy(v_h[:], v_sb[:])
        VS = []
        for ec in range(2):
            t = sing.tile([128, 31], F16, tag=f"VS{ec}")
            nc.gpsimd.memset(t[:], 0.0)
            nc.vector.tensor_copy(t[:, 15:16], v_h[:, ec : ec + 1])
            VS.append(t)

        emit_xT(1)
        emit_us_x(1, 0)
        emit_WxT(1, 0)
        emit_us_x(1, 1)
        emit_WxT(1, 1)

        # ---------------- one-time LSTM weight prep ----------------
        # gate order in pytorch rows: i,f,g,o -> slot i:0 f:1 o:2 g:3
        SLOT = {0: 0, 1: 1, 2: 3, 3: 2}
        wihT = [[None] * DC for _ in range(2)]  # [dr][dc] [128, 4*H] f16
        whhT = [None, None]  # [dr] [128, 4*H] f16
        biasT = [None, None]  # [dr] [4, 128] f32 (row gc)
        ones1 = None
        bones = None

        def emit_lstm_prep():
            nonlocal ones1, bones
            for dr in range(2):
                for dc in range(DC):
                    pst = ps_tr.tile([128, 512], F32, tag="tr")
                    for gc in range(4):
                        nc.tensor.transpose(
                            pst[:, gc * 128 : (gc + 1) * 128],
                            wih_sb[dr][:, gc * D + dc * 128 : gc * D + (dc + 1) * 128],
                            ident[:],
                        )
                    t = sing.tile([128, 4 * H], F16, tag=f"wihT{dr}{dc}")
                    nc.vector.tensor_copy(t[:], pst[:])
                    # sigmoid gates (i,f,o = cols of gc 0,1,3) pre-scaled 0.5
                    nc.vector.tensor_scalar_mul(t[:, 0:256], t[:, 0:256], 0.5)
                    nc.vector.tensor_scalar_mul(t[:, 384:512], t[:, 384:512], 0.5)
                    wihT[dr][dc] = t
                pst = ps_tr.tile([128, 512], F32, tag="tr")
                for gc in range(4):
                    nc.tensor.transpose(
                        pst[:, gc * 128 : (gc + 1) * 128],
                        whh_sb[dr][:, gc * H : (gc + 1) * H],
                        ident[:],
                    )
                t = sing.tile([128, 4 * H], F16, tag=f"whhT{dr}")
                # h carried as 2h -> all whh gates 0.5x; sigmoid gates 0.25x
                nc.vector.tensor_copy(t[:], pst[:])
                nc.vector.tensor_scalar_mul(t[:, 0:256], t[:, 0:256], 0.25)
                nc.vector.tensor_scalar_mul(t[:, 256:384], t[:, 256:384], 0.5)
                nc.vector.tensor_scalar_mul(t[:, 384:512], t[:, 384:512], 0.25)
                whhT[dr] = t
                # bias: (bih+bhh) transposed to [1, gc*128+h], sigmoid gates 0.5x
                bsum = sing.tile([128, 4], F32, tag=f"bsum{dr}")
                nc.vector.tensor_add(bsum[:], bias_raw[dr][0][:], bias_raw[dr][1][:])
                psb = ps_sm.tile([1, 512], F32, tag="sm")
                for gc in range(4):
                    nc.tensor.transpose(
                        psb[:, gc * 128 : (gc + 1) * 128],
                        bsum[:, gc : gc + 1],
                        ident[:],
                    )
                bt = sing.tile([1, 512], F32, tag=f"biasT{dr}")
                nc.vector.tensor_copy(bt[:], psb[:])
                nc.vector.tensor_scalar_mul(bt[:, 0:256], bt[:, 0:256], 0.5)
                nc.vector.tensor_scalar_mul(bt[:, 384:512], bt[:, 384:512], 0.5)
                biasT[dr] = bt
            ones1 = sing.tile([1, 4], F32, tag="ones1")
            nc.gpsimd.memset(ones1[:], 1.0)
            # bones[k=(b,pos,oct), c=(b,pos)] = 1 iff k//4 == c
            bones = sing.tile([16, 4], F32, tag="bones")
            nc.gpsimd.memset(bones[:], 1.0)
            nc.gpsimd.affine_select(
                out=bones[:], in_=bones[:], pattern=[[4, 4]],
                channel_multiplier=-1, base=3, compare_op=ALU.is_ge, fill=0.0,
            )
            nc.gpsimd.affine_select(
                out=bones[:], in_=bones[:], pattern=[[-4, 4]],
                channel_multiplier=1, base=0, compare_op=ALU.is_ge, fill=0.0,
            )

        # ---------------- per-pair state ----------------
        scP = [None] * NPAIR  # [16, 512] psum tiles
        abarT_sb = sing.tile([L, BL * N], F32, tag="abarT")  # col = b*8+n
        uT = [sing.tile([128, BL * N], F16, tag=f"uT{dc}", name=f"uT{dc}") for dc in range(DC)]
        gxb = sing.tile([128, 128], F32, tag="gxb")  # col = dr*64+sl*16+b*8+n
        hseqT = [sing.tile([128, BL * N], F16, tag=f"hseqT{dc}", name=f"hseqT{dc}") for dc in range(DC)]
        zeros_h = sing.tile([128, 2], F16, tag="zeros_h")
        nc.gpsimd.memset(zeros_h[:], 0.0)
        zeros4 = sing.tile([128, 4], F32, tag="zeros4")
        nc.gpsimd.memset(zeros4[:], 0.0)
        lstm_state = {"h": [zeros_h[:], zeros_h[:]], "c": zeros4[:]}

        def emit_tile(p, b, ec, chain_start, chain_stop):
            """e-add + tanh + 8 shifted-v score matmuls for (pair p, b, ec)."""
            e_t = ep.tile([128, 2 * SL], F16, tag="e")
            in0 = vw(
                WxT_h[ec][:, b * N * L + p * L :],
                [[(7 - 2 * p) * L, 2], [0, S], [1, L]],
            )
            in1 = vw(us_x[(b, ec)][:], [[0, 2], [L, S], [1, L]])
            nc.vector.tensor_add(
                e_t[:].rearrange("p (a s l) -> p a s l", a=2, s=S), in0, in1
            )
            t_t = tp.tile([128, 2 * SL], F16, tag="t")
            nc.scalar.activation(t_t[:], e_t[:], AF.Tanh)
            for pos in range(2):
                for oct_ in range(4):
                    t = b * 8 + pos * 4 + oct_
                    first = chain_start and pos == 0 and oct_ == 0
                    last = chain_stop and pos == 1 and oct_ == 3
                    nc.tensor.matmul(
                        scP[p][:],
                        VS[ec][:, 15 - t : 31 - t],
                        t_t[:, pos * SL + oct_ * 512 : pos * SL + oct_ * 512 + 512],
                        start=first,
                        stop=last,
                    )

        def emit_pair_tail(p):
            """softmax -> abar -> abarT -> uT -> gxb for pair p."""
            es = esp.tile([16, SL // 4], F32, tag="es")
            nc.scalar.activation(es[:], scP[p][:], AF.Exp)
            den = lsm.tile([16, 8], F32, tag="den")
            nc.vector.reduce_sum(
                den[:], es[:].rearrange("p (s l) -> p s l", s=8), axis=AX.X
            )
            rden = lsm.tile([16, 8], F32, tag="rden")
            nc.vector.reciprocal(rden[:], den[:])
            nc.vector.tensor_scalar_mul(rden[:], rden[:], 1.0 / S)
            nc.vector.tensor_mul(
                es[:].rearrange("p (s l) -> p s l", s=8),
                es[:].rearrange("p (s l) -> p s l", s=8),
                vw(rden[:, 0:1], [[1, 8], [0, L]]),
            )
            abarp = lsm.tile([16, L], F32, tag="abarp")
            nc.vector.reduce_sum(
                abarp[:], vw(es[:, 0:1], [[1, L], [L, 8]]), axis=AX.X
            )
            # abarT[l, (b,pos)] = sum_oct abarp[(b,pos,oct), l]
            psa = ps_sm.tile([L, 4], F32, tag="sm")
            nc.tensor.matmul(psa[:], abarp[:], bones[:], start=True, stop=True)
            nc.vector.tensor_copy(
                vw(abarT_sb[:, p : p + 1], [[8, 2], [7 - 2 * p, 2]]), psa[:]
            )
            # uT[dc][:, bn] for the 4 (b,pos) of this pair
            psu = ps_sm.tile([128, 8], F32, tag="sm")
            for dc in range(DC):
                for i, bn in enumerate(
                    (b * N + nn for b in range(BL) for nn in (p, 7 - p))
                ):
                    nc.tensor.matmul(
                        psu[:, dc * 4 + i : dc * 4 + i + 1],
                        x_nat[:, bn * D + dc * 128 : bn * D + (dc + 1) * 128],
                        abarT_sb[:, bn : bn + 1],
                        start=True,
                        stop=True,
                    )
            for dc in range(DC):
                nc.vector.tensor_copy(
                    vw(uT[dc][:, p : p + 1], [[8, 2], [7 - 2 * p, 2]]),
                    psu[:, dc * 4 : dc * 4 + 4],
                )
            # gxb: per dr accumulate wih @ uT + bias (rank-1) into [128,16]
            for dr in range(2):
                gp = ps_sm.tile([128, 16], F32, tag="sm")
                for gc in range(4):
                    sl = SLOT[gc]
                    for dc in range(DC):
                        nc.tensor.matmul(
                            gp[:, sl * 4 : sl * 4 + 4],
                            wihT[dr][dc][:, gc * 128 : (gc + 1) * 128],
                            vw(uT[dc][:, p : p + 1], [[8, 2], [7 - 2 * p, 2]]),
                            start=(dc == 0),
                            stop=False,
                        )
                    nc.tensor.matmul(
                        gp[:, sl * 4 : sl * 4 + 4],
                        biasT[dr][0:1, gc * 128 : (gc + 1) * 128],
                        ones1[:],
                        start=False,
                        stop=True,
                    )
                nc.vector.tensor_copy(
                    vw(
                        gxb[:, dr * 64 + p : dr * 64 + p + 1],
                        [[16, 4], [8, 2], [7 - 2 * p, 2]],
                    ),
                    gp[:].rearrange("p (sl b o) -> p sl b o", sl=4, b=2),
                )

        def emit_lstm_step(k):
            """One BiLSTM step; gates via single tanh (sigmoid = tanh-half)."""
            h_prev, c_prev = lstm_state["h"], lstm_state["c"]
            ghp = ps_sm.tile([128, 16], F32, tag="sm")
            # ghp col = dr*8 + sl*2 + b
            for dr in range(2):
                for gc in range(4):
                    sl = SLOT[gc]
                    nc.tensor.matmul(
                        ghp[:, dr * 8 + sl * 2 : dr * 8 + sl * 2 + 2],
                        whhT[dr][:, gc * 128 : (gc + 1) * 128],
                        h_prev[dr],
                        start=True,
                        stop=True,
                    )
            pf, pb = k, N - 1 - k
            # gx view matching ghp col order (dr, sl, b)
            gx = vw(
                gxb[:, pf : pf + 1],
                [[64 + pb - pf, 2], [16, 4], [8, 2]],
            )
            gsum = lsm.tile([128, 16], F32, tag="gsum")
            nc.vector.tensor_add(
                gsum[:].rearrange("p (d g b) -> p d g b", d=2, g=4),
                ghp[:].rearrange("p (d g b) -> p d g b", d=2, g=4),
                gx,
            )
            sg = lsm.tile([128, 16], F32, tag="sg")
            nc.scalar.activation(sg[:], gsum[:], AF.Tanh)
            # t1 = (tf+1)*c_prev = 2*f*c ; t2 = (ti+1)*tg = 2*i*g
            t1 = lsm.tile([128, 4], F32, tag="t1")
            nc.vector.scalar_tensor_tensor(
                t1[:], vw(sg[:, 2:3], [[8, 2], [1, 2]]), 1.0, c_prev,
                op0=ALU.add, op1=ALU.mult,
            )
            t2 = lsm.tile([128, 4], F32, tag="t2")
            nc.vector.scalar_tensor_tensor(
                t2[:], vw(sg[:, 0:1], [[8, 2], [1, 2]]), 1.0,
                vw(sg[:, 6:7], [[8, 2], [1, 2]]),
                op0=ALU.add, op1=ALU.mult,
            )
            c_dbl = lsm.tile([128, 4], F32, tag="cdbl")
            nc.vector.tensor_add(c_dbl[:], t1[:], t2[:])  # = 2*c_new
            tc_ = lsm.tile([128, 4], F32, tag="tc")
            nc.scalar.activation(tc_[:], c_dbl[:], AF.Tanh, scale=0.5)
            c_new = lsm.tile([128, 4], F32, tag="c")
            nc.vector.tensor_scalar_mul(c_new[:], c_dbl[:], 0.5)
            for dr in range(2):
                pos = k if dr == 0 else N - 1 - k
                hv = vw(hseqT[dr][:, pos : pos + 1], [[8, 2]])
                # h2 = (to+1)*tanh(c) = 2*o*tanh(c); o-gate cols dr*8+4+b
                nc.vector.scalar_tensor_tensor(
                    hv, vw(sg[:, dr * 8 + 4 : dr * 8 + 5], [[1, 2]]),
                    1.0, tc_[:, dr * 2 : dr * 2 + 2],
                    op0=ALU.add, op1=ALU.mult,
                )
                lstm_state["h"][dr] = hv
            lstm_state["c"] = c_new[:]

        # ---------------- stage C pipeline ----------------
        UNLOCK = {0: [0], 1: [1], 2: [2], 3: [3, 4, 5, 6, 7]}
        for p in range(NPAIR):
            scP[p] = ps_sc.tile([16, 512], F32, tag="sc", name=f"scP{p}")
            first_blk = True
            for b in range(BL):
                for ec in range(2):
                    emit_tile(p, b, ec, chain_start=(b == 0 and ec == 0),
                              chain_stop=(b == BL - 1 and ec == 1))
                    if first_blk:
                        first_blk = False
                        if p == 0:
                            emit_lstm_prep()
                        else:
                            emit_pair_tail(p - 1)
                            for k in UNLOCK[p - 1]:
                                emit_lstm_step(k)
        emit_pair_tail(NPAIR - 1)
        for k in UNLOCK[NPAIR - 1]:
            emit_lstm_step(k)

        # ---------------- stage G: attention pooling ----------------
        saWT = [sing.tile([128, D], F16, tag=f"saWT{dc}", name=f"saWT{dc}") for dc in range(DC)]
        for dc in range(DC):
            pst = ps_tr.tile([128, 512], F32, tag="tr")
            for ec in range(2):
                nc.tensor.transpose(
                    pst[:, ec * 128 : (ec + 1) * 128],
                    saW_sb[:, ec * D + dc * 128 : ec * D + (dc + 1) * 128],
                    ident[:],
                )
            t = saWT[dc]
            nc.vector.tensor_copy(t[:], pst[:, 0:D])
            # h carried as 2h -> saW pre-scaled 0.5
            nc.vector.tensor_scalar_mul(t[:], t[:], 0.5)

        th = []
        for ec in range(2):
            ap_ps = ps_sm.tile([128, BL * N], F32, tag="sm")
            for dc in range(DC):
                nc.tensor.matmul(
                    ap_ps[:],
                    saWT[dc][:, ec * 128 : (ec + 1) * 128],
                    hseqT[dc][:],
                    start=(dc == 0),
                    stop=(dc == DC - 1),
                )
            t = sing.tile([128, BL * N], F32, tag=f"th{ec}")
            nc.scalar.activation(
                t[:], ap_ps[:], AF.Tanh, bias=sab_sb[:, ec : ec + 1]
            )
            th.append(t)
        att_ps = ps_sm.tile([1, BL * N], F32, tag="sm")
        for ec in range(2):
            nc.tensor.matmul(
                att_ps[:], sav_sb[:, ec : ec + 1], th[ec][:],
                start=(ec == 0), stop=(ec == 1),
            )
        att_sb = sing.tile([1, BL * N], F32, tag="att_sb")
        nc.vector.tensor_copy(att_sb[:], att_ps[:])
        attv = att_sb[:].rearrange("p (b n) -> p b n", b=BL)
        mxa = sing.tile([1, BL], F32, tag="mxa")
        nc.vector.reduce_max(mxa[:], attv, axis=AX.X)
        nc.vector.tensor_sub(attv, attv, vw(mxa[:], [[1, BL], [0, N]]))
        nc.scalar.activation(att_sb[:], att_sb[:], AF.Exp)
        dena = sing.tile([1, BL], F32, tag="dena")
        nc.vector.reduce_sum(dena[:], attv, axis=AX.X)
        rdena = sing.tile([1, BL], F32, tag="rdena")
        nc.vector.reciprocal(rdena[:], dena[:])
        # h2 carry -> att pre-scaled 0.5
        nc.vector.tensor_scalar_mul(rdena[:], rdena[:], 0.5)
        nc.vector.tensor_mul(attv, attv, vw(rdena[:], [[1, BL], [0, N]]))
        attT = sing.tile([BL * N, 1], F32, tag="attT")
        psa = ps_sm.tile([BL * N, 1], F32, tag="sm")
        nc.tensor.transpose(psa[:], att_sb[:], ident[0:1, 0:1])
        nc.vector.tensor_copy(attT[:], psa[:])
        c_nat = sing.tile([BL * N, D], F32, tag="c_nat")
        ident_h = sing.tile([128, 128], F16, tag="ident_h")
        nc.vector.tensor_copy(ident_h[:], ident[:])
        for dc in range(DC):
            pst = ps_tr.tile([128, 512], F16, tag="tr")
            nc.tensor.transpose(pst[0 : BL * N, 0:128], hseqT[dc][:], ident_h[:])
            nc.vector.tensor_copy(
                c_nat[:, dc * 128 : (dc + 1) * 128], pst[0 : BL * N, 0:128]
            )
        cw = sing.tile([BL * N, D], F32, tag="cw")
        nc.vector.tensor_scalar_mul(cw[:], c_nat[:], attT[:, 0:1])
        ones_blk = sing.tile([BL * N, BL], F32, tag="ones_blk")
        nc.gpsimd.memset(ones_blk[:], 1.0)
        nc.gpsimd.affine_select(
            out=ones_blk[:], in_=ones_blk[:], pattern=[[N, BL]],
            channel_multiplier=-1, base=N - 1, compare_op=ALU.is_ge, fill=0.0,
        )
        nc.gpsimd.affine_select(
            out=ones_blk[:], in_=ones_blk[:], pattern=[[-N, BL]],
            channel_multiplier=1, base=0, compare_op=ALU.is_ge, fill=0.0,
        )
        g_ps = ps_sm.tile([BL, D], F32, tag="sm")
        nc.tensor.matmul(g_ps[:], ones_blk[:], cw[:], start=True, stop=True)
        g_sb = sing.tile([BL, D], F32, tag="g_sb")
        nc.vector.tensor_copy(g_sb[:], g_ps[:])
        nc.sync.dma_start(g_d.ap()[:], g_sb[:])

    split_sync_waits(nc)
    return nc


_NC = None


def kernel(**inputs) -> np.ndarray:
    global _NC
    if _NC is None:
        _NC = build()
    f32 = lambda a: np.ascontiguousarray(np.asarray(a, dtype=np.float32))
    shared = {
        k: f32(inputs[k])
        for k in (
            "W", "U", "v",
            "Wih_f", "Whh_f", "bih_f", "bhh_f",
            "Wih_b", "Whh_b", "bih_b", "bhh_b",
            "saW", "sab", "sav",
        )
    }
    x = f32(inputs["x"])
    s = f32(inputs["s"])
    in_maps = []
    for c in range(NCORES):
        m = dict(shared)
        m["x"] = np.ascontiguousarray(x[c * BL : (c + 1) * BL])
        m["s"] = np.ascontiguousarray(s[c * BL : (c + 1) * BL])
        in_maps.append(m)
    res = run_bass_kernel_spmd(_NC, in_maps, core_ids=list(range(NCORES)))
    return np.concatenate([r["g"] for r in res.results], axis=0)
